# revision 26
# baseline (speedup 1.0000x reference)
"""Trainium2 Bass kernel for nn_Distribution_74758200754679.

Computes, for x [65536, 8, 256] and a tiny MLP (256 -> 128 -> 1):
    h    = leaky_relu(x @ W1 + b1, 0.3)
    beta = sigmoid(h @ W2 + b2)            # [B, N]
    p    = stick_breaking(beta)            # [B, N+1]

Distribution: pure data parallel over 8 NeuronCores — x is sharded along
the batch axis, MLP params are replicated. Each core's shard is staged
host-side in transposed fp16 layout (d_in on partitions) so the device
loop is a chain of full-rate fp16 matmuls with no on-chip transpose and
half the HBM traffic of fp32.

Per-core device program (32 MB of x per core, 128 blocks x 512 rows),
processed in PAIRS of blocks so consecutive matmuls share stationary
weights (fewer PE LDWEIGHTS/instruction-decode stalls):
  DMA xT chunks (ramped sizes so compute starts early)
  -> L1: LD(W1_lo) MM(even) MM(odd), LD(W1_hi) MM(even) MM(odd),
     K=256 accumulated in PSUM
  -> leaky relu in ONE op (alternating ACT Prelu / DVE max(0.3z, z));
     L2 of pair p is emitted after the L1s of pair p+1 (software
     pipelining) so the PE never waits on the leaky ops
  -> L2: one shared [128, 32] stationary (sliding window of a [128, 63]
     tile whose col 31 holds W2) + two matmuls into PSUM bank A (even
     blocks) and bank B (odd blocks); 32 pairs accumulate their beta
     rows into distinct partitions of the two [32, 512] PSUM banks
  -> per-bank tail (runs under the PE stream): sigmoid straight from
     PSUM + suffix-product stick-breaking + that bank's output DMA.
"""

import os
import sys

# The device path runs through jax/PJRT on the neuron (axon) platform; a
# cpu-pinned JAX_PLATFORMS would hide the NeuronCores.
if os.environ.get("JAX_PLATFORMS") == "cpu":
    os.environ["JAX_PLATFORMS"] = ""

for _p in ("/opt/trn_rl_repo",):
    if _p not in sys.path:
        sys.path.insert(0, _p)

import numpy as np
from contextlib import ExitStack

import concourse.bacc as bacc
import concourse.mybir as mybir
from concourse import tile
from concourse import bass_utils

B, N, D_IN, D_H = 65536, 8, 256, 128
SLOPE = 0.3
CORES = 8
RC = B * N // CORES          # rows per core (65536)
BC = B // CORES              # batches per core (8192)
BLK = 512                    # rows per block
NBLK = RC // BLK             # 128
NG = BLK // N                # batch groups per partition in the tail (64)
SGRP = 64                    # blocks per beta supergroup (2 PSUM banks)
NPAIR = SGRP // 2            # pairs per supergroup (32)
# x DMA chunk sizes in blocks: small first chunks so the PE starts while
# the DMA engines are still ramping
CHUNKS = [2, 2, 4] + [8] * 15
assert sum(CHUNKS) == NBLK

f32 = mybir.dt.float32
f32r = mybir.dt.float32r
f16 = mybir.dt.float16
AF = mybir.ActivationFunctionType
ALU = mybir.AluOpType

_NC_CACHE = {}
_LAST_RESULTS = None


def _build(use_bias: bool):
    nc = bacc.Bacc(
        "TRN2", target_bir_lowering=False, debug=False, num_devices=CORES
    )
    xt_d = nc.dram_tensor("xt", [D_IN, RC], f16, kind="ExternalInput").ap()
    w1_d = nc.dram_tensor("w1", [D_IN, D_H], f16, kind="ExternalInput").ap()
    # sliding-window W2 stationary: col 31 holds W2, zeros elsewhere;
    # pair p uses cols [31-p, 63-p) -> W2 sits at in-window index p
    w2s_d = nc.dram_tensor("w2s", [D_H, 2 * NPAIR - 1], f16, kind="ExternalInput").ap()
    b1c_d = nc.dram_tensor("b1c", [D_H, 1], f32, kind="ExternalInput").ap()
    st_d = nc.dram_tensor("st", [128, 1], f32, kind="ExternalInput").ap()
    nst_d = nc.dram_tensor("nst", [128, 1], f32, kind="ExternalInput").ap()
    p_d = nc.dram_tensor("p", [BC, N + 1], f32, kind="ExternalOutput").ap()

    with tile.TileContext(nc) as tc, ExitStack() as ctx:
        const = ctx.enter_context(tc.tile_pool(name="const", bufs=1))
        xpool = ctx.enter_context(tc.tile_pool(name="xp", bufs=1))
        hpool = ctx.enter_context(tc.tile_pool(name="hp", bufs=1))
        tpool = ctx.enter_context(tc.tile_pool(name="tp", bufs=1))
        psh = ctx.enter_context(tc.tile_pool(name="psh", bufs=1, space="PSUM"))
        psb = ctx.enter_context(tc.tile_pool(name="psb", bufs=1, space="PSUM"))

        def T(pool, shape, dt_, nm, bufs=1):
            tag = nm.split("_")[0]
            return pool.tile(shape, dt_, name=nm, tag=tag, bufs=bufs)

        w1_sb = T(const, [128, 2, D_H], f16, "w1sb")
        nc.sync.dma_start(w1_sb[:], w1_d.rearrange("(kc p) m -> p kc m", kc=2))
        w2s_sb = T(const, [D_H, 2 * NPAIR - 1], f16, "w2ssb")
        nc.sync.dma_start(w2s_sb[:], w2s_d[:])
        b1c_sb = T(const, [D_H, 1], f32, "b1csb")
        nc.sync.dma_start(b1c_sb[:], b1c_d[:])
        st_sb = T(const, [128, 1], f32, "stsb")
        nc.sync.dma_start(st_sb[:], st_d[:])
        nst_sb = T(const, [128, 1], f32, "nstsb")
        nc.sync.dma_start(nst_sb[:], nst_d[:])

        def tail_group(sgrp, member, pb):
            """Stick-breaking for one PSUM bank of 32 blocks' betas.

            Bank partition p holds beta for block sgrp*SGRP + 2*p + member.
            Runs under the PE stream except for the very last bank.
            """
            g = 2 * sgrp + member
            sg = T(tpool, [NPAIR, BLK], f32, f"sg_{g}")
            nc.scalar.activation(
                sg[:], pb[:], AF.Sigmoid, bias=st_sb[0:NPAIR, :], scale=1.0
            )
            gg = T(tpool, [NPAIR, BLK], f32, f"gg_{g}")  # 1 - beta
            nc.scalar.activation(
                gg[:], pb[:], AF.Sigmoid, bias=nst_sb[0:NPAIR, :], scale=-1.0
            )
            # suffix products s[e] = prod_{k>=e} gg[k] via in-place
            # log-tree: s[0:N-k] *= s[k:N] (forward refs are safe)
            s = T(tpool, [NPAIR, BLK], f32, f"s_{g}")
            nc.vector.tensor_copy(s[:], gg[:])
            sv = s[:].rearrange("p (gr e) -> p gr e", e=N)
            for k in (1, 2, 4):
                nc.vector.tensor_mul(
                    sv[:, :, 0:N - k], sv[:, :, 0:N - k], sv[:, :, k:N]
                )
            # P[gr*9]   = s[gr*8]                    (p[b, 0])
            # P[gr*9+i] = beta[i-1] * s[i], i=1..7;  P[gr*9+8] = beta[7]
            P = T(tpool, [NPAIR, NG * (N + 1)], f32, f"P_{g}")
            Pv = P[:].rearrange("p (gr e) -> p gr e", e=N + 1)
            sgv = sg[:].rearrange("p (gr e) -> p gr e", e=N)
            nc.vector.tensor_copy(Pv[:, :, 0:1], sv[:, :, 0:1])
            nc.vector.tensor_mul(Pv[:, :, 1:N], sgv[:, :, 0:N - 1], sv[:, :, 1:N])
            nc.vector.tensor_copy(Pv[:, :, N:N + 1], sgv[:, :, N - 1:N])
            # partition p -> block sgrp*64 + 2p + member -> 64 batches
            rows = SGRP * NG  # 4096 batches per supergroup
            dest = p_d[sgrp * rows:(sgrp + 1) * rows, :].rearrange(
                "(pr two gr) e -> two pr (gr e)", two=2, gr=NG
            )[member]
            nc.sync.dma_start(dest, P[:])

        # software pipelining: the L2 matmuls of pair p are emitted after
        # the L1 matmuls of pair p+1, so the PE never waits on the leaky
        # activations of the pair it just produced.
        pend = None  # (hhE, hhO, pairidx, pbA, pbB)

        def emit_l2(pd_):
            hhE, hhO, pi, pbA_, pbB_ = pd_
            sgrp, p = pi // NPAIR, pi % NPAIR
            stat = w2s_sb[:, NPAIR - 1 - p:2 * NPAIR - 1 - p]
            nc.tensor.matmul(
                pbA_[:], stat, hhE[:],
                start=(p == 0), stop=(p == NPAIR - 1),
            )
            nc.tensor.matmul(
                pbB_[:], stat, hhO[:],
                start=(p == 0), stop=(p == NPAIR - 1),
            )
            if p == NPAIR - 1:
                tail_group(sgrp, 0, pbA_)
                tail_group(sgrp, 1, pbB_)

        pbA = pbB = None
        blk0 = 0
        for ci, cblocks in enumerate(CHUNKS):
            dcols = cblocks * BLK
            c0 = blk0 * BLK
            bufs = 1 if cblocks < 8 else 4
            x0 = T(xpool, [128, dcols], f16, f"x0c{cblocks}_{ci}", bufs=bufs)
            nc.sync.dma_start(x0[:], xt_d[0:128, c0:c0 + dcols])
            x1 = T(xpool, [128, dcols], f16, f"x1c{cblocks}_{ci}", bufs=bufs)
            nc.sync.dma_start(x1[:], xt_d[128:256, c0:c0 + dcols])
            for sub in range(0, cblocks, 2):
                blk = blk0 + sub           # even member of the pair
                pi = blk // 2              # global pair index
                if blk % SGRP == 0:
                    sg_i = blk // SGRP
                    pbA = T(psb, [NPAIR, BLK], f32, f"pbA_{sg_i}", bufs=2)
                    pbB = T(psb, [NPAIR, BLK], f32, f"pbB_{sg_i}", bufs=2)
                csE = slice(sub * BLK, (sub + 1) * BLK)
                csO = slice((sub + 1) * BLK, (sub + 2) * BLK)

                phE = T(psh, [128, BLK], f32, f"phE_{blk}", bufs=2)
                phO = T(psh, [128, BLK], f32, f"phO_{blk}", bufs=2)
                nc.tensor.matmul(phE[:], w1_sb[:, 0, :], x0[:, csE], start=True, stop=False)
                nc.tensor.matmul(phO[:], w1_sb[:, 0, :], x0[:, csO], start=True, stop=False)
                nc.tensor.matmul(phE[:], w1_sb[:, 1, :], x1[:, csE], start=False, stop=True)
                nc.tensor.matmul(phO[:], w1_sb[:, 1, :], x1[:, csO], start=False, stop=True)
                if pend is not None:
                    emit_l2(pend)
                    pend = None

                # even block: ACT Prelu (one op, PSUM read)
                hhE = T(hpool, [128, BLK], f16, f"hhE_{blk}", bufs=3)
                nc.scalar.activation(
                    hhE[:], phE[:], AF.Prelu,
                    bias=b1c_sb[:], scale=1.0, alpha=SLOPE,
                )
                # odd block: leaky via DVE; only one DVE input may read
                # PSUM, so stage z in SBUF first
                hhO = T(hpool, [128, BLK], f16, f"hhO_{blk}", bufs=3)
                if use_bias:
                    nc.scalar.activation(
                        hhO[:], phO[:], AF.Prelu,
                        bias=b1c_sb[:], scale=1.0, alpha=SLOPE,
                    )
                else:
                    zc = T(hpool, [128, BLK], f16, f"zc_{blk}", bufs=3)
                    nc.vector.tensor_copy(zc[:], phO[:])
                    nc.vector.scalar_tensor_tensor(
                        hhO[:], zc[:], SLOPE, zc[:], op0=ALU.mult, op1=ALU.max
                    )
                pend = (hhE, hhO, pi, pbA, pbB)
            blk0 += cblocks
        emit_l2(pend)
        pend = None

    nc.compile()
    return nc


def _get_nc(use_bias: bool = False):
    if use_bias not in _NC_CACHE:
        _NC_CACHE[use_bias] = _build(use_bias)
    return _NC_CACHE[use_bias]


def kernel(**inputs):
    x = np.asarray(inputs["x"], dtype=np.float32)
    W1 = np.ascontiguousarray(
        np.asarray(inputs["W1"], dtype=np.float32).astype(np.float16)
    )
    b1 = np.asarray(inputs["b1"], dtype=np.float32)
    W2 = np.ascontiguousarray(np.asarray(inputs["W2"], dtype=np.float32))
    b2 = np.asarray(inputs["b2"], dtype=np.float32)

    use_bias = bool(np.any(b1 != 0.0))
    nc = _get_nc(use_bias)

    xf = x.reshape(B * N, D_IN)
    w2s = np.zeros((D_H, 2 * NPAIR - 1), np.float16)
    w2s[:, NPAIR - 1] = W2[:, 0].astype(np.float16)
    b1c = np.ascontiguousarray(b1.reshape(D_H, 1))
    stv = np.full((128, 1), np.float32(b2[0]), np.float32)
    nstv = np.ascontiguousarray(-stv)

    in_maps = []
    for c in range(CORES):
        shard = xf[c * RC:(c + 1) * RC]
        xt = shard.T.astype(np.float16)      # [256, RC], fp16 halves HBM traffic
        in_maps.append({
            "xt": xt, "w1": W1, "w2s": w2s,
            "b1c": b1c, "st": stv, "nst": nstv,
        })

    res = bass_utils.run_bass_kernel_spmd(
        nc, in_maps, core_ids=list(range(CORES))
    )
    global _LAST_RESULTS
    _LAST_RESULTS = res
    p = np.concatenate(
        [res.results[c]["p"] for c in range(CORES)], axis=0
    ).astype(np.float32)
    return p
